# revision 4
# baseline (speedup 1.0000x reference)
"""DBRX MoE experts kernel for 8 Trainium2 NeuronCores (expert-parallel).

Strategy:
  - Host: router matmul + softmax + top-4 + renormalize (tiny: 0.27 GFLOP),
    gather tokens per expert, pre-transpose/re-tile all operands.
  - Device (SPMD, 8 cores, 2 experts each): per expert, SwiGLU FFN over its
    gathered tokens in float32r (TF32-like PE mode, full speed, ~1e-4 rel err):
       GT[i,c] = W1T.T@XgT, UT[i,c] = V1T.T@XgT  (accumulate over d)
       HT[i,c] = silu(GT)*UT                      (ACT + DVE)
       YT[d,c] = W2T.T@HT                         (accumulate over i)
    Everything is computed transposed ([feature, token]) so the chain needs
    no on-chip transposes. Tokens are processed in 2 passes to fit HT in SBUF
    (weights stream twice; DMA stays under PE time).
  - Host: scale rows by gates and scatter-add into the output.
"""
import sys
sys.path.insert(0, "/opt/trn_rl_repo")
import numpy as np

import concourse.bass as bass
import concourse.mybir as mybir
import concourse.tile as tile
import concourse.tile_sem_assignment as _tsa

# This walrus build only supports ONE sync-wait command per instruction.
# Route all HWDGE DMA completions through a single semaphore so consumers
# need at most one DMA wait...
_tsa.NUM_HWDGE_SEMS = 1

N_CORES = 8
E = 16
E_LOC = 2
D = 2048
I = 4096
TOP_K = 4
NDT = D // 128   # 16 d-tiles
NIT = I // 128   # 32 i-tiles

_F32R = mybir.dt.float32r
_F32 = mybir.dt.float32


def _split_multi_waits(nc):
    """...and split any instruction that still carries >1 sync wait into
    single-wait EventSemaphore prefixes (semantically identical: waits are
    ANDed, the sequencer executes them in order before the instruction)."""
    ctr = 0
    for f in nc.m.functions:
        for blk in f.blocks:
            insts = list(blk.instructions)
            out = []
            changed = False
            for inst in insts:
                si = inst.sync_info
                if si is not None and si.on_wait is not None and len(si.on_wait) > 1:
                    waits = list(si.on_wait)
                    for w in waits[:-1]:
                        ctr += 1
                        out.append(mybir.InstEventSemaphore(
                            name=f"wsplit_{ctr}",
                            engine=inst.engine,
                            ins=[], outs=[],
                            sync_info=mybir.SyncInfo(on_wait=[w], on_update=[]),
                            bass_nofuse=True,
                        ))
                    inst.sync_info = mybir.SyncInfo(
                        on_wait=[waits[-1]], on_update=list(si.on_update or []))
                    changed = True
                out.append(inst)
            if changed:
                blk.instructions.clear()
                for i2 in out:
                    blk.add_instruction(i2)


def _chunks(n):
    """Split even-length [0, n) into even-sized PSUM-bank chunks (<=512 each,
    >=256 when possible; fp32r matmuls need an even moving free dim)."""
    assert n % 2 == 0
    if n <= 512:
        return [(0, n)]
    k = -(-n // 512)
    sizes = [(n // k) & ~1] * k
    rem, j = n - sum(sizes), 0
    while rem > 0:
        sizes[j] += 2
        rem -= 2
        j = (j + 1) % k
    out, s = [], 0
    for sz in sizes:
        out.append((s, sz))
        s += sz
    return out


def _build_nc(C, Cp_list, rep=1):
    """One SPMD program; per-core inputs differ only in data."""
    nc = bass.Bass(target_bir_lowering=False)
    xt_d = nc.dram_tensor("xt", [E_LOC, NDT, 128, C], _F32R, kind="ExternalInput")
    wv1_d = nc.dram_tensor("wv1", [E_LOC, NIT, 128, 2 * NDT * 128], _F32R,
                           kind="ExternalInput")
    w2_d = nc.dram_tensor("w2", [E_LOC, NDT, 128, NIT * 128], _F32R,
                          kind="ExternalInput")
    yt_d = nc.dram_tensor("yt", [E_LOC, NDT, 128, C], _F32, kind="ExternalOutput")

    with tile.TileContext(nc) as tc:
        with (
            tc.tile_pool(name="xt", bufs=2) as xt_pool,
            tc.tile_pool(name="ht", bufs=1) as ht_pool,
            tc.tile_pool(name="wg", bufs=2) as wg_pool,
            tc.tile_pool(name="ev", bufs=2) as ev_pool,
            tc.tile_pool(name="ps", bufs=2, space="PSUM") as ps,
        ):
            for rp in range(rep):
                for el in range(E_LOC):
                    for (plo, phi) in Cp_list:
                        _emit_expert_pass(nc, tc, xt_pool, ht_pool, wg_pool,
                                          ev_pool, ps, xt_d, wv1_d, w2_d, yt_d,
                                          rp, el, plo, phi)
    nc.finalize()
    _split_multi_waits(nc)
    return nc


def _emit_expert_pass(nc, tc, xt_pool, ht_pool, wg_pool, ev_pool, ps,
                      xt_d, wv1_d, w2_d, yt_d, rp, el, plo, phi):
    Cp = phi - plo
    ch = _chunks(Cp)
    tagbase = f"{rp}_{el}_{plo}"
    # --- load this pass's token tiles (one packed tile) ---
    xts = xt_pool.tile([128, NDT, Cp], _F32R, tag="xt", name=f"xt_{tagbase}")
    for dt in range(NDT):
        nc.sync.dma_start(xts[:, dt, :], xt_d[el, dt, :, plo:phi])
    # --- stage 1+2: HT[it] = silu(W1T.T@X) * (V1T.T@X) ---
    hts = ht_pool.tile([128, NIT, Cp], _F32R, tag="ht", name=f"ht_{tagbase}")
    for it in range(NIT):
        wv = wg_pool.tile([128, 2, NDT, 128], _F32R, tag="wg",
                          name=f"wv_{tagbase}_{it}")
        nc.sync.dma_start(
            wv[:], wv1_d[el, it].rearrange("p (w t i) -> p w t i", w=2, t=NDT))
        gs, us = [], []
        for ci, (c0, cn) in enumerate(ch):
            gt = ps.tile([128, cn], _F32, tag=f"pg{ci}",
                         name=f"g{ci}_{tagbase}_{it}")
            ut = ps.tile([128, cn], _F32, tag=f"pu{ci}",
                         name=f"u{ci}_{tagbase}_{it}")
            gs.append(gt)
            us.append(ut)
        for w in range(2):
            pts = gs if w == 0 else us
            for dt in range(NDT):
                for ci, (c0, cn) in enumerate(ch):
                    nc.tensor.matmul(
                        pts[ci][:], wv[:, w, dt, :], xts[:, dt, c0:c0 + cn],
                        start=(dt == 0), stop=(dt == NDT - 1))
        hs = ev_pool.tile([128, Cp], _F32, tag="hs", name=f"hs_{tagbase}_{it}")
        for ci, (c0, cn) in enumerate(ch):
            nc.scalar.activation(hs[:, c0:c0 + cn], gs[ci][:],
                                 mybir.ActivationFunctionType.Silu)
            nc.vector.tensor_tensor(
                out=hts[:, it, c0:c0 + cn], in0=us[ci][:],
                in1=hs[:, c0:c0 + cn], op=mybir.AluOpType.mult)
    # --- stage 3: YT[dt] = W2T.T @ HT ---
    for dt in range(NDT):
        w2 = wg_pool.tile([128, NIT, 128], _F32R, tag="wg",
                          name=f"w2_{tagbase}_{dt}")
        nc.sync.dma_start(
            w2[:], w2_d[el, dt].rearrange("p (t i) -> p t i", t=NIT))
        ys = []
        for ci, (c0, cn) in enumerate(ch):
            yt = ps.tile([128, cn], _F32, tag=f"pg{ci}",
                         name=f"y{ci}_{tagbase}_{dt}")
            ys.append(yt)
        for it in range(NIT):
            for ci, (c0, cn) in enumerate(ch):
                nc.tensor.matmul(
                    ys[ci][:], w2[:, it, :], hts[:, it, c0:c0 + cn],
                    start=(it == 0), stop=(it == NIT - 1))
        yo = ev_pool.tile([128, Cp], _F32, tag="yo", name=f"yo_{tagbase}_{dt}")
        for ci, (c0, cn) in enumerate(ch):
            nc.scalar.activation(yo[:, c0:c0 + cn], ys[ci][:],
                                 mybir.ActivationFunctionType.Copy)
        nc.sync.dma_start(yt_d[el, dt, :, plo:phi], yo[:])


def _prepare(hidden_states, router_w, ws, w2s, rep=1):
    hs = np.ascontiguousarray(hidden_states, dtype=np.float32)
    rw = np.ascontiguousarray(router_w, dtype=np.float32)
    ws = np.asarray(ws, dtype=np.float32)
    w2s = np.asarray(w2s, dtype=np.float32)
    T, D_ = hs.shape
    assert (D_, ws.shape[0], ws.shape[1], w2s.shape[1], w2s.shape[2]) == \
        (D, E, 2 * I, D, I), "kernel compiled for DBRX 16x(2048->4096) shapes"

    # ---- routing on host (softmax -> top-4 -> renormalize) ----
    logits = hs @ rw.T                                   # [T, E]
    m = logits.max(axis=-1, keepdims=True)
    p = np.exp(logits - m)
    p /= p.sum(axis=-1, keepdims=True)
    topk_idx = np.argpartition(-p, TOP_K - 1, axis=-1)[:, :TOP_K]   # [T, 4]
    topk_val = np.take_along_axis(p, topk_idx, axis=-1)
    gates_w = topk_val / topk_val.sum(axis=-1, keepdims=True)

    tok_idx, tok_gate = [None] * E, [None] * E
    flat_e = topk_idx.ravel()
    flat_g = gates_w.ravel()
    flat_t = np.repeat(np.arange(T), TOP_K)
    order = np.argsort(flat_e, kind="stable")
    se, st, sg = flat_e[order], flat_t[order], flat_g[order]
    bounds = np.searchsorted(se, np.arange(E + 1))
    for e in range(E):
        tok_idx[e] = st[bounds[e]:bounds[e + 1]]
        tok_gate[e] = sg[bounds[e]:bounds[e + 1]]

    C = max(4, int(max(len(t) for t in tok_idx)))
    C = -(-C // 4) * 4  # multiple of 4: both token passes stay even-length
    # two token passes so HT fits in SBUF
    half = C // 2
    Cp_list = [(0, half), (half, C)]

    # ---- build per-core inputs ----
    in_maps = []
    for c in range(N_CORES):
        xt = np.zeros((E_LOC, NDT, 128, C), dtype=np.float32)
        wv1 = np.empty((E_LOC, NIT, 128, 2 * NDT * 128), dtype=np.float32)
        w2 = np.empty((E_LOC, NDT, 128, NIT * 128), dtype=np.float32)
        for el in range(E_LOC):
            e = c * E_LOC + el
            xg = hs[tok_idx[e]]                          # [n_e, D]
            # xt[el, dt, p, c] = xg[c, dt*128+p]
            xt[el, :, :, :len(tok_idx[e])] = xg.T.reshape(NDT, 128, -1)
            w1 = ws[e, :I, :]                            # [I, D]
            v1 = ws[e, I:, :]
            # wv1[el, it, p, (w, dt, ii)] = {w1,v1}[it*128+ii, dt*128+p]
            wv = np.stack([w1, v1]).reshape(2, NIT, 128, NDT, 128)
            wv = wv.transpose(1, 4, 0, 3, 2)             # [it, p, w, dt, ii]
            wv1[el] = np.ascontiguousarray(wv).reshape(NIT, 128, -1)
            # w2[el, dt, p, (it, ii)] = w2s[e, dt*128+ii, it*128+p]
            w2e = w2s[e].reshape(NDT, 128, NIT, 128)     # [dt, ii, it, p]
            w2[el] = np.ascontiguousarray(
                w2e.transpose(0, 3, 2, 1)).reshape(NDT, 128, -1)
        in_maps.append({"xt": xt, "wv1": wv1, "w2": w2})

    def combine(results):
        out = np.zeros((T, D), dtype=np.float32)
        for c in range(N_CORES):
            yt = results[c]["yt"]                        # [E_LOC, NDT, 128, C]
            for el in range(E_LOC):
                e = c * E_LOC + el
                n_e = len(tok_idx[e])
                if n_e == 0:
                    continue
                y = yt[el].reshape(D, C)[:, :n_e].T      # [n_e, D]
                out[tok_idx[e]] += tok_gate[e][:, None].astype(np.float32) * y
        return out

    nc = _build_nc(C, Cp_list, rep=rep)
    return {"nc": nc, "in_maps": in_maps, "combine": combine, "C": C}


def kernel(hidden_states, router_w, ws, w2s):
    from concourse.bass_utils import run_bass_kernel_spmd
    prep = _prepare(hidden_states, router_w, ws, w2s)
    res = run_bass_kernel_spmd(prep["nc"], prep["in_maps"],
                               core_ids=list(range(N_CORES)))
    return prep["combine"](res.results)
